# revision 1
# baseline (speedup 1.0000x reference)
"""Trainium2 Bass kernel for nn_CnUpdateLayer (LDPC check-node update).

Math: out[b,i] = prod_{j: mask[i,j]!=0} x[b,j], or 0 if mask row i is empty.
Mask is {0,1} and x ~ randn (no exact zeros), so the masked product is
computed in log-domain via one matmul pass:

    L[b,i] = sum_j ln(x[b,j]^2) * mask[i,j]      (magnitude, log domain)
    C[b,i] = sum_j [x[b,j]<0]   * mask[i,j]      (negative count)
    deg[i] = sum_j mask[i,j]                     (row degree)
    out    = exp(0.5*L) * (min(deg,1) - 2*(C mod 2))

The stationary operand W = [sgn | mag | ones | 0] is exactly 128 bf16/fp16
columns (enables fast-weight-load); mag = fp16(ln(x^2)) gives ~5e-3 worst
rel err (vs the 2e-2 gate).  C mod 2 uses fp32 round-to-nearest-even at the
2-ulp grid: p = C - ((C + 2^24) - 2^24) in {0,+-1}, p^2 = C mod 2.  PSUM
row layout puts C on partitions 0-31 so the whole DVE parity chain is
partition-aligned; only ACT does cross-partition reads (Exp on 32-63,
degree copy on 64-95), which the scalar engine supports.

Perf structure per core (tensor-parallel over output edges, no collectives):
  - single ACT table load: a hand-placed LoadActFuncSet of the
    natural_log_exp_and_others set covers Ln AND Exp, so the compiler pass
    inserts no further (1.3us each) reloads.
  - inputs: x pre-cast fp16 (128KB), mask shard pre-cast fp8e4 (512KB),
    pre-swizzled so every DMA is contiguous per partition.  Each HWDGE
    ring's first transfer lands ~2us before its second, so x heads the
    sync ring, mask half-2 heads the scalar ring, mask half-1 rides
    second on sync -- and the matmuls consume chunks 8-15 first so the
    late half is also the last consumed (accumulation commutes).
  - a gap-free train of dummy matmuls (memset operand, scratch PSUM bank)
    keeps the PE busy from engine-start so the HAM clock-gate opens
    (1.2 -> 2.4 GHz) before/while the 16 real accumulation matmuls issue;
    any >0.5us gap in the train resets the HAM busy window.
"""

import sys

if "/opt/trn_rl_repo" not in sys.path:
    sys.path.insert(0, "/opt/trn_rl_repo")

import numpy as np

B = 32          # batch codewords
IN_F = 2048     # input edges
OUT_F = 2048    # output edges
NCORES = 8
SHARD = OUT_F // NCORES     # 256 output edges per core
KC = IN_F // 128            # 16 contraction chunks of 128
# W column layout (PSUM reads must start 32-partition aligned)
WSGN, WMAG, WONE, WPAD = 0, B, 2 * B, 3 * B    # 0, 32, 64, 96
WTOT = 128
MAGIC = float(2 ** 24)

_PROG = None


def _build_program():
    import concourse.tile as tile
    from concourse import bacc, mybir
    from concourse.alu_op_type import AluOpType

    F32 = mybir.dt.float32
    F16 = mybir.dt.float16
    BF16 = mybir.dt.bfloat16
    FP8 = mybir.dt.float8e4
    AF = mybir.ActivationFunctionType

    nc = bacc.Bacc("TRN2", target_bir_lowering=False)
    xt = nc.dram_tensor("xt", [128, KC * B], F16, kind="ExternalInput")
    mt = nc.dram_tensor("mt", [128, KC * SHARD], FP8, kind="ExternalInput")
    out = nc.dram_tensor("out", [B, SHARD], F32, kind="ExternalOutput")

    with tile.TileContext(nc) as tc:
        with (
            tc.tile_pool(name="pool", bufs=1) as pool,
            tc.tile_pool(name="psum", bufs=1, space="PSUM") as psum_pool,
        ):
            # ---- single ACT table load: natural_log_exp_and_others (id 6)
            # covers Ln and Exp, so bacc's insert_act_table_loads sees every
            # activation already resident and inserts no reloads.  Must be
            # FIRST in the scalar stream: placed mid-stream, the pass
            # inserts its own extra load ahead of it.
            nc.scalar.add_instruction(mybir.InstLoadActFuncSet(
                name=nc.get_next_instruction_name(), act_func_set_id=6,
                engine=mybir.EngineType.Activation, ins=[], outs=[]))

            # ---- PE warm-up: dummy matmuls on a memset operand into a
            # scratch PSUM bank.  PE busy time opens the HAM clock gate
            # (1.2 -> 2.4 GHz) so the real matmuls run at the warm issue
            # rate.
            # One memset, one dummy shape: the PE busy train must be GAP-FREE
            # (a >0.5us gap resets the HAM busy window and the real matmuls
            # run at 1.2 GHz).  N=256 dummies start ~0.3us earlier than
            # N=512 ones behind a full-tile memset.
            dmy = pool.tile([128, 256], BF16)
            nc.vector.memset(dmy, 1.0)
            ps_warm = psum_pool.tile([128, 256], F32)
            for _ in range(14):
                nc.tensor.matmul(ps_warm, lhsT=dmy[:, 0:128], rhs=dmy,
                                 start=True, stop=True)

            # ---- input DMAs.  Each ring's FIRST transfer lands ~2us sooner
            # than its second, so: x whole heads the sync ring, mask half-2
            # heads the scalar ring (lands ~9.2us), mask half-1 rides second
            # on sync.  The matmuls consume chunks 8-15 FIRST (accumulation
            # commutes), so the late half is also the last consumed.
            x_sb = pool.tile([128, KC, B], F16)
            xt_v = xt.ap().rearrange("p (c b) -> p c b", b=B)
            m_sb = pool.tile([128, KC, SHARD], FP8)
            mt_v = mt.ap().rearrange("p (c n) -> p c n", n=SHARD)
            HG = KC // 2
            nc.sync.dma_start(out=x_sb, in_=xt_v)
            nc.scalar.dma_start(out=m_sb[:, HG:KC, :], in_=mt_v[:, HG:KC, :])
            nc.sync.dma_start(out=m_sb[:, 0:HG, :], in_=mt_v[:, 0:HG, :])

            # ---- stationary operand W = [sgn | mag | ones | 0], fp16.
            # mag = ln(x^2) (x^2 on DVE avoids an Abs pass; the 0.5 folds
            # into the Exp scale), cast to fp16 directly by the ACT output.
            # sgn also on DVE: GpSimd is ~10x slower on this op and locks
            # DVE out of the shared SBUF port for the whole instruction.
            w_sb = pool.tile([128, KC, WTOT], F16)
            nc.vector.memset(w_sb[:, :, WONE:WONE + B], 1.0)
            nc.vector.memset(w_sb[:, :, WPAD:WPAD + B], 0.0)
            sq_sb = pool.tile([128, KC, B], F32)
            for h in (HG, 0):              # second half first: its mask half
                sl = slice(h, h + HG)      # lands first, so it is consumed first
                nc.vector.tensor_tensor(
                    out=sq_sb[:, sl, :], in0=x_sb[:, sl, :], in1=x_sb[:, sl, :],
                    op=AluOpType.mult)
                nc.scalar.activation(
                    out=w_sb[:, sl, WMAG:WMAG + B], in_=sq_sb[:, sl, :], func=AF.Ln)
                nc.vector.tensor_scalar(
                    out=w_sb[:, sl, WSGN:WSGN + B], in0=x_sb[:, sl, :],
                    scalar1=0.0, scalar2=None, op0=AluOpType.is_lt)

            # ---- main accumulation: ps += W_c^T @ M_c over 16 chunks,
            # chunks 8-15 first (their mask half lands first) ----
            ps = psum_pool.tile([128, SHARD], F32)
            order = list(range(HG, KC)) + list(range(0, HG))
            for i, c in enumerate(order):
                nc.tensor.matmul(
                    ps, lhsT=w_sb[:, c, :], rhs=m_sb[:, c, :],
                    start=(i == 0), stop=(i == KC - 1))

            # ---- epilogue ----
            # The PSUM bank tracker serializes accessors of the ps bank
            # pairwise in trace order, so evacuate all 96 live rows with ONE
            # DVE copy (partition-aligned read, base 0); everything
            # downstream reads SBUF and the engines run in parallel.
            csb = pool.tile([3 * B, SHARD], F32)
            nc.vector.tensor_scalar(
                out=csb, in0=ps[0:3 * B, :], scalar1=0.0, scalar2=None,
                op0=AluOpType.add)
            cC = csb[WSGN:WSGN + B, :]     # negative counts, partitions 0-31

            # ACT (cross-partition reads are fine on the scalar engine):
            # e = relu(1 - deg) = [deg == 0]; a = exp(0.5*L).  e first: the
            # DVE chain needs e (via v) one op before it needs a.
            e = pool.tile([B, SHARD], F32)
            nc.scalar.activation(out=e, in_=csb[WONE:WONE + B, :], func=AF.Relu,
                                 scale=-1.0, bias=1.0)
            a = pool.tile([B, SHARD], F32)
            nc.scalar.activation(out=a, in_=csb[WMAG:WMAG + B, :], func=AF.Exp, scale=0.5)

            # DVE parity chain, partition-aligned on 0-31 (mod fails the
            # walrus ISA check on DVE, so parity goes through fp32
            # round-to-nearest-even at the 2-ulp grid):
            # t = fp32(C + 2^24) = 2^24 + rne2(C), q = [(t - 2^24) != C]
            # = C mod 2; out = a * (1 - e - 2*q).
            t = pool.tile([B, SHARD], F32)
            nc.vector.tensor_scalar(
                out=t, in0=cC, scalar1=MAGIC, scalar2=None, op0=AluOpType.add)
            q = pool.tile([B, SHARD], F32)
            nc.vector.scalar_tensor_tensor(
                out=q, in0=t, scalar=MAGIC, in1=cC,
                op0=AluOpType.subtract, op1=AluOpType.not_equal)
            v = pool.tile([B, SHARD], F32)
            nc.vector.scalar_tensor_tensor(
                out=v, in0=q, scalar=-2.0, in1=e,
                op0=AluOpType.mult, op1=AluOpType.subtract)
            o_sb = pool.tile([B, SHARD], F32)
            nc.vector.scalar_tensor_tensor(
                out=o_sb, in0=v, scalar=1.0, in1=a,
                op0=AluOpType.add, op1=AluOpType.mult)
            nc.sync.dma_start(out=out.ap(), in_=o_sb)

    nc.compile()
    return nc


def _get_program():
    global _PROG
    if _PROG is None:
        _PROG = _build_program()
    return _PROG


def _prep_inputs(x, mask):
    import ml_dtypes

    x = np.ascontiguousarray(x, dtype=np.float32)
    mask = np.ascontiguousarray(mask, dtype=np.float32)
    # xt[p, c*B + b] = x[b, c*128 + p], fp16
    xt = np.ascontiguousarray(
        x.T.reshape(KC, 128, B).transpose(1, 0, 2).reshape(128, KC * B)
    ).astype(np.float16)
    mask_f8 = mask.astype(ml_dtypes.float8_e4m3)      # 0/1: exact
    in_maps = []
    for k in range(NCORES):
        shard = mask_f8[k * SHARD:(k + 1) * SHARD, :]      # [256, 2048]
        # mt[p, c*SHARD + n] = mask[k*SHARD + n, c*128 + p]
        mt = np.ascontiguousarray(
            shard.T.reshape(KC, 128, SHARD).transpose(1, 0, 2).reshape(128, KC * SHARD))
        in_maps.append({"xt": xt, "mt": mt})
    return in_maps


def run(x, mask, trace=False):
    """Run on 8 NeuronCores; returns (output, BassKernelResults)."""
    from concourse.bass_utils import run_bass_kernel_spmd

    nc = _get_program()
    in_maps = _prep_inputs(x, mask)
    res = run_bass_kernel_spmd(nc, in_maps, core_ids=list(range(NCORES)), trace=trace)
    out = np.concatenate([r["out"] for r in res.results], axis=1)
    return np.ascontiguousarray(out, dtype=np.float32), res


def kernel(x, mask):
    out, _ = run(x, mask, trace=False)
    return out

